# revision 1
# baseline (speedup 1.0000x reference)
"""Trainium2 Bass kernel for nn_GRNNTransformGated (recursive tree GRNN over
1024 independent 10-level binary jets).

Strategy:
  - Data-parallel over jets: 8 cores x 128 trees each.
  - Canonical children layout (node k -> children 2k, 2k+1) means child
    "gathers" are stride-2 slices of the level below; the whole bottom-up
    recursion stays in SBUF (only contents in, root embeddings out).
  - Feature-major layout [128 features (partitions), nodes (free)] so every
    matmul is lhsT.T @ rhs with weight blocks stationary.
  - conv_chain collapses: for w>0, b>=0, f(f(f(x))) = w^2*relu(w*x+b) + (w*b+b).
  - sigmoid via tanh (same ACT table set as exp): r = 0.5*(tanh(q/2)+1); the
    0.5 is folded into W_h on the host.
  - softmax reciprocal on the vector engine (exp+recip can't share an ACT
    table set).
"""

import sys

for _p in ("/opt/trn_rl_repo", "/root/.axon_site/_ro/trn_rl_repo"):
    if _p not in sys.path:
        sys.path.insert(0, _p)

import numpy as np

B = 1024
L = 10
H = 128
FEAT = 7
NCORES = 8
TPC = B // NCORES          # trees per core = 128
TCH = 16                   # trees per chunk
NCHUNK = TPC // TCH        # 8 chunks
NPC = TPC * (2 ** L - 1)   # nodes per core = 130944
LOFF = [TPC * (2 ** j - 1) for j in range(L + 1)]  # level offsets in per-core ct
LEVEL_SIZES = [B * 2 ** j for j in range(L)]
OFF = np.concatenate([[0], np.cumsum(LEVEL_SIZES)]).astype(int)
INNER = LEVEL_SIZES[:-1]
COFF = np.concatenate([[0], np.cumsum(INNER)]).astype(int)

MMT = 512  # matmul node-tile size

_CACHE = {}


def _children_canonical(children):
    for j in range(L - 1):
        n = INNER[j]
        blk = children[COFF[j]:COFF[j + 1]]
        base = 2 * np.arange(n, dtype=np.int64)
        if not (np.array_equal(blk[:, 0], base) and np.array_equal(blk[:, 1], base + 1)):
            return False
    return True


def _numpy_fallback(contents, children, W_u, b_u, W_h, b_h, W_z, b_z, W_r, b_r,
                    conv_w, conv_b):
    w, b = float(conv_w[0]), float(conv_b[0])

    def conv_chain(x):
        for _ in range(3):
            x = np.maximum(w * x + b, 0.0)
        return x

    def sigmoid(x):
        return 1.0 / (1.0 + np.exp(-x))

    emb = None
    for j in reversed(range(L)):
        c = contents[OFF[j]:OFF[j + 1]]
        u = conv_chain(c @ W_u + b_u)
        if j == L - 1:
            emb = u
            continue
        ch = children[COFF[j]:COFF[j + 1]]
        h_L = emb[ch[:, 0]]
        h_R = emb[ch[:, 1]]
        hhu = np.concatenate([h_L, h_R, u], axis=1)
        r = sigmoid(hhu @ W_r + b_r)
        h_H = conv_chain((r * hhu) @ W_h + b_h)
        z = np.concatenate([h_H, hhu], axis=1) @ W_z + b_z
        zs = np.stack([z[:, :H], z[:, H:2 * H], z[:, 2 * H:3 * H], z[:, 3 * H:]], axis=-1)
        zs = zs - zs.max(axis=-1, keepdims=True)
        e = np.exp(zs)
        g = e / e.sum(axis=-1, keepdims=True)
        emb = g[..., 0] * h_H + g[..., 1] * h_L + g[..., 2] * h_R + g[..., 3] * u
    return emb.reshape(B, -1).astype(np.float32)


def _build(cw, cb, collapsible, do_affine, A, C):
    from contextlib import ExitStack

    from concourse import bacc, bass, mybir, tile

    f32 = mybir.dt.float32
    bf16 = mybir.dt.bfloat16
    AF = mybir.ActivationFunctionType
    OP = mybir.AluOpType

    nc = bacc.Bacc()

    ct_d = nc.declare_dram_parameter("ct", [FEAT, NPC], bf16, isOutput=False)
    wu_d = nc.declare_dram_parameter("wu", [FEAT, H], bf16, isOutput=False)
    wr_d = nc.declare_dram_parameter("wr", [H, 3, 3, H], bf16, isOutput=False)
    wh_d = nc.declare_dram_parameter("wh", [H, 3, H], bf16, isOutput=False)
    wz_d = nc.declare_dram_parameter("wz", [H, 4, 4, H], bf16, isOutput=False)
    bv_d = nc.declare_dram_parameter("bvec", [H, 9], f32, isOutput=False)
    id_d = nc.declare_dram_parameter("ident", [H, H], f32, isOutput=False)
    out_d = nc.declare_dram_parameter("out", [TPC, H], f32, isOutput=True)

    with ExitStack() as ctx:
        tc = ctx.enter_context(tile.TileContext(nc))
        wpool = ctx.enter_context(tc.tile_pool(name="wts", bufs=1))
        epool = ctx.enter_context(tc.tile_pool(name="emb", bufs=1))
        ctpool = ctx.enter_context(tc.tile_pool(name="ct", bufs=3))
        spool = ctx.enter_context(tc.tile_pool(name="tmp", bufs=2))
        upool = ctx.enter_context(tc.tile_pool(name="utmp", bufs=3))
        ppu = ctx.enter_context(tc.tile_pool(name="ppu", bufs=1, space="PSUM"))
        ppr = ctx.enter_context(tc.tile_pool(name="ppr", bufs=1, space="PSUM"))
        ppz = ctx.enter_context(tc.tile_pool(name="ppz", bufs=1, space="PSUM"))

        wu = wpool.tile([FEAT, H], bf16, tag="wu")
        wr = wpool.tile([H, 3, 3, H], bf16, tag="wr")
        wh = wpool.tile([H, 3, H], bf16, tag="wh")
        wz = wpool.tile([H, 4, 4, H], bf16, tag="wz")
        bv = wpool.tile([H, 9], f32, tag="bv")
        idt = wpool.tile([H, H], f32, tag="idt")
        nc.sync.dma_start(wu[:], wu_d[:])
        nc.sync.dma_start(wr[:], wr_d[:])
        nc.sync.dma_start(wh[:], wh_d[:])
        nc.sync.dma_start(wz[:], wz_d[:])
        nc.sync.dma_start(bv[:], bv_d[:])
        nc.sync.dma_start(idt[:], id_d[:])

        # emb level buffers (phase A holds one chunk; emb5 accumulates all chunks)
        e9 = epool.tile([H, TCH * 512], bf16, tag="e9")     # 8192
        e8 = epool.tile([H, TCH * 256], bf16, tag="e8")     # 4096
        e7 = epool.tile([H, TCH * 128], bf16, tag="e7")     # 2048
        e6 = epool.tile([H, TCH * 64], bf16, tag="e6")      # 1024
        emb5 = epool.tile([H, TPC * 32], bf16, tag="emb5")  # 4096 (all trees)

        def conv_emit(dst, psum_ap, bias_col):
            """dst = conv_chain(psum + b_lin) with b_lin folded in bias col."""
            nc.scalar.activation(dst, psum_ap, AF.Relu, bias=bv[:, bias_col:bias_col + 1],
                                 scale=cw)
            if collapsible:
                if do_affine:
                    nc.vector.tensor_scalar(dst, dst, A, C, OP.mult, OP.add)
            else:
                nc.scalar.activation(dst, dst, AF.Relu, bias=cb, scale=cw)
                nc.scalar.activation(dst, dst, AF.Relu, bias=cb, scale=cw)

        def inner_tile(cb3, pbase, ct_ap, out_ap, n):
            """One tile of n parent nodes at offset pbase within the level."""
            hL = cb3[:, pbase:pbase + n, 0]
            hR = cb3[:, pbase:pbase + n, 1]
            # ---- u ----
            pu = ppu.tile([H, MMT], f32, name="puh", tag="puh")
            nc.tensor.matmul(pu[:, :n], wu[:], ct_ap, start=True, stop=True)
            up = upool.tile([H, MMT], bf16, name="up", tag="up")
            u = up[:, :n]
            conv_emit(u, pu[:, :n], 0)
            rhs_k = [hL, hR, u]
            # ---- r gates (as tanh) ----
            prs = [ppr.tile([H, MMT], f32, name=f"pr{m}", tag=f"pr{m}") for m in range(3)]
            for m in range(3):
                for k in range(3):
                    nc.tensor.matmul(prs[m][:, :n], wr[:, k, m, :], rhs_k[k],
                                     start=(k == 0), stop=(k == 2))
            tts = []
            for m in range(3):
                tm = spool.tile([H, MMT], f32, name=f"t{m}", tag=f"t{m}")
                nc.scalar.activation(tm[:, :n], prs[m][:, :n], AF.Tanh,
                                     bias=bv[:, 1 + m:2 + m], scale=0.5)
                tts.append(tm)
            # ---- rh = (t+1) * hhu   (x0.5 folded into W_h) ----
            rhs_h = []
            for k in range(3):
                rk = spool.tile([H, MMT], bf16, name=f"rh{k}", tag=f"rh{k}")
                nc.vector.scalar_tensor_tensor(rk[:, :n], tts[k][:, :n], 1.0,
                                               rhs_k[k], OP.add, OP.mult)
                rhs_h.append(rk)
            ph = ppu.tile([H, MMT], f32, name="puh", tag="puh")
            for k in range(3):
                nc.tensor.matmul(ph[:, :n], wh[:, k, :], rhs_h[k][:, :n],
                                 start=(k == 0), stop=(k == 2))
            hp = upool.tile([H, MMT], bf16, name="hp", tag="hp")
            hH = hp[:, :n]
            conv_emit(hH, ph[:, :n], 4)
            # ---- z ----
            zk = [hH, hL, hR, u]
            pzs = [ppz.tile([H, MMT], f32, name=f"pz{m}", tag=f"pz{m}") for m in range(4)]
            for m in range(4):
                for k in range(4):
                    nc.tensor.matmul(pzs[m][:, :n], wz[:, k, m, :], zk[k],
                                     start=(k == 0), stop=(k == 3))
            es = []
            for m in range(4):
                em = spool.tile([H, MMT], f32, name=f"e{m}", tag=f"e{m}")
                nc.scalar.activation(em[:, :n], pzs[m][:, :n], AF.Exp,
                                     bias=bv[:, 5 + m:6 + m])
                es.append(em)
            e0, e1, e2, e3 = [e[:, :n] for e in es]
            # ---- softmax-weighted combine ----
            s01 = spool.tile([H, MMT], f32, name="s01", tag="s01")
            s23 = spool.tile([H, MMT], f32, name="s23", tag="s23")
            nc.vector.tensor_tensor(s01[:, :n], e0, e1, OP.add)
            nc.gpsimd.tensor_tensor(s23[:, :n], e2, e3, OP.add)
            nc.vector.tensor_tensor(s01[:, :n], s01[:, :n], s23[:, :n], OP.add)
            rcp = spool.tile([H, MMT], f32, name="rcp", tag="rcp")
            nc.vector.reciprocal_approx_fast(rcp[:, :n], s01[:, :n])
            nc.vector.tensor_tensor(e0, e0, hH, OP.mult)
            nc.gpsimd.tensor_tensor(e1, e1, hL, OP.mult)
            nc.gpsimd.tensor_tensor(e2, e2, hR, OP.mult)
            nc.gpsimd.tensor_tensor(e3, e3, u, OP.mult)
            nc.vector.tensor_tensor(e0, e0, e1, OP.add)
            nc.vector.tensor_tensor(e2, e2, e3, OP.add)
            nc.vector.tensor_tensor(e0, e0, e2, OP.add)
            nc.vector.tensor_tensor(out_ap, e0, rcp[:, :n], OP.mult)

        def run_level(nj, ct_base, cbuf, obuf_ap):
            """One level with nj parents; children in cbuf (2*nj wide)."""
            cb3 = cbuf[:].rearrange("p (n two) -> p n two", two=2)
            done = 0
            while done < nj:
                piece = min(2048, nj - done)
                ctt = ctpool.tile([FEAT, 2048], bf16, name="ctt", tag="ctt")
                nc.sync.dma_start(ctt[:, :piece],
                                  ct_d[:, ct_base + done:ct_base + done + piece])
                for s in range(0, piece, MMT):
                    n = min(MMT, piece - s)
                    pbase = done + s
                    inner_tile(cb3, pbase, ctt[:, s:s + n],
                               obuf_ap[:, pbase:pbase + n], n)
                done += piece

        # ================= phase A: per-chunk levels 9..5 =================
        for c in range(NCHUNK):
            # leaf level 9
            nleaf = TCH * 512  # 8192
            base9 = LOFF[9] + c * nleaf
            for hpiece in range(0, nleaf, 2048):
                ctt = ctpool.tile([FEAT, 2048], bf16, name="ctt", tag="ctt")
                nc.sync.dma_start(ctt[:], ct_d[:, base9 + hpiece:base9 + hpiece + 2048])
                for s in range(0, 2048, MMT):
                    pu = ppu.tile([H, MMT], f32, name="puh", tag="puh")
                    nc.tensor.matmul(pu[:], wu[:], ctt[:, s:s + MMT],
                                     start=True, stop=True)
                    dst = e9[:, hpiece + s:hpiece + s + MMT]
                    nc.scalar.activation(dst, pu[:], AF.Relu,
                                         bias=bv[:, 0:1], scale=cw)
                    if not collapsible:
                        nc.scalar.activation(dst, dst, AF.Relu, bias=cb, scale=cw)
                        nc.scalar.activation(dst, dst, AF.Relu, bias=cb, scale=cw)
                if collapsible and do_affine:
                    big = e9[:, hpiece:hpiece + 2048]
                    nc.vector.tensor_scalar(big, big, A, C, OP.mult, OP.add)
            # inner levels 8..5
            for j, (cbuf, obuf) in zip(
                    range(8, 4, -1),
                    [(e9, e8), (e8, e7), (e7, e6), (e6, None)]):
                nj = TCH * (2 ** j)
                if j == 5:
                    ob = emb5[:, c * 512:(c + 1) * 512]
                else:
                    ob = obuf[:, :nj]
                run_level(nj, LOFF[j] + c * nj, cbuf, ob)

        # ================= phase B: levels 4..0, all trees =================
        # reuse dead phase-A buffers for the tail levels
        e4 = e8[:, :2048]
        e3 = e7[:, :1024]
        e2 = e6[:, :512]
        e1 = e8[:, 2048:2048 + 256]
        e0f = epool.tile([H, TPC], f32, tag="e0f")
        e0 = e0f[:, :TPC]
        chain = [(emb5[:], e4), (e4, e3), (e3, e2), (e2, e1), (e1, e0)]
        for j, (cbap, ob) in zip(range(4, -1, -1), chain):
            nj = TPC * (2 ** j)
            cb3v = cbap.rearrange("p (n two) -> p n two", two=2)
            done = 0
            while done < nj:
                piece = min(2048, nj - done)
                ctt = ctpool.tile([FEAT, 2048], bf16, name="ctt", tag="ctt")
                nc.sync.dma_start(ctt[:, :piece],
                                  ct_d[:, LOFF[j] + done:LOFF[j] + done + piece])
                for s in range(0, piece, MMT):
                    n = min(MMT, piece - s)
                    pbase = done + s
                    inner_tile(cb3v, pbase, ctt[:, s:s + n], ob[:, pbase:pbase + n], n)
                done += piece

        # ================= output transpose + store =================
        pt = ppz.tile([H, H], f32, name="pz0", tag="pz0")
        nc.tensor.matmul(pt[:], e0, idt[:], is_transpose=True, start=True, stop=True)
        osb = spool.tile([H, H], f32, name="osb", tag="osb")
        nc.vector.tensor_copy(osb[:], pt[:])
        nc.sync.dma_start(out_d[:], osb[:])

    nc.compile()
    if not nc.is_finalized():
        nc.finalize()
    return nc


def _prepare(inputs):
    contents = np.ascontiguousarray(np.asarray(inputs["contents"], np.float32))
    W_u = np.asarray(inputs["W_u"], np.float32)
    b_u = np.asarray(inputs["b_u"], np.float32)
    W_h = np.asarray(inputs["W_h"], np.float32)
    b_h = np.asarray(inputs["b_h"], np.float32)
    W_z = np.asarray(inputs["W_z"], np.float32)
    b_z = np.asarray(inputs["b_z"], np.float32)
    W_r = np.asarray(inputs["W_r"], np.float32)
    b_r = np.asarray(inputs["b_r"], np.float32)
    cw = float(np.asarray(inputs["conv_w"]).reshape(-1)[0])
    cb = float(np.asarray(inputs["conv_b"]).reshape(-1)[0])

    # per-core feature-major contents, level-major columns
    cts = np.empty((NCORES, FEAT, NPC), np.float32)
    col = 0
    for j in range(L):
        n = TPC * 2 ** j
        blk = contents[OFF[j]:OFF[j + 1]].reshape(NCORES, n, FEAT)
        cts[:, :, col:col + n] = blk.transpose(0, 2, 1)
        col += n

    wr_np = np.ascontiguousarray(W_r.reshape(3, H, 3, H).transpose(1, 0, 2, 3))
    wz_np = np.ascontiguousarray(W_z.reshape(4, H, 4, H).transpose(1, 0, 2, 3))
    wh_np = np.ascontiguousarray((0.5 * W_h).reshape(3, H, H).transpose(1, 0, 2))

    bvec = np.zeros((H, 9), np.float32)
    bvec[:, 0] = cw * b_u + cb
    bvec[:, 1:4] = 0.5 * b_r.reshape(3, H).T
    bvec[:, 4] = cw * b_h + cb
    bvec[:, 5:9] = b_z.reshape(4, H).T

    import ml_dtypes

    bf = ml_dtypes.bfloat16
    common = {
        "wu": np.ascontiguousarray(W_u).astype(bf),
        "wr": wr_np.astype(bf), "wh": wh_np.astype(bf), "wz": wz_np.astype(bf),
        "bvec": bvec,
        "ident": np.eye(H, dtype=np.float32),
    }
    in_maps = [dict(common, ct=np.ascontiguousarray(cts[c]).astype(bf))
               for c in range(NCORES)]
    return in_maps, cw, cb


def kernel(**inputs):
    children = np.asarray(inputs["children"])
    cw = float(np.asarray(inputs["conv_w"]).reshape(-1)[0])
    cb = float(np.asarray(inputs["conv_b"]).reshape(-1)[0])
    collapsible = (cw >= 0.0) and (cb >= 0.0)
    if not _children_canonical(children):
        args = {k: np.asarray(v) for k, v in inputs.items()}
        return _numpy_fallback(**args)

    from concourse.bass_utils import run_bass_kernel_spmd

    A = cw * cw
    C = cw * cb + cb
    do_affine = not (A == 1.0 and C == 0.0)

    key = (cw, cb, collapsible, do_affine)
    if key not in _CACHE:
        _CACHE[key] = _build(cw, cb, collapsible, do_affine, A, C)
    nc = _CACHE[key]

    in_maps, _, _ = _prepare(inputs)
    res = run_bass_kernel_spmd(nc, in_maps, list(range(NCORES)))
    outs = [res.results[c]["out"] for c in range(NCORES)]
    return np.ascontiguousarray(np.concatenate(outs, axis=0).astype(np.float32))


if __name__ == "__main__":
    rng = np.random.default_rng(0)
    print("kernel module loaded")



# revision 2
# speedup vs baseline: 888.2264x; 888.2264x over previous
"""Trainium2 Bass kernel for nn_GRNNTransformGated (recursive tree GRNN over
1024 independent 10-level binary jets) — restructured for speed.

Key ideas vs baseline:
  - De-interleaved (Ev|Od) embedding storage per level: child gathers become
    contiguous slices; all DVE ops run packed bf16 (2x/4x perf modes).
  - z3-normalized softmax: z'_m = z_m - z_3 folded into W_z host-side.
    12 z matmul passes instead of 16, 3 exps instead of 4, e3 == 1.
  - conv_chain(y) = relu(w^3 y + w^2 b) + C fully folded: w^3 into W_u/W_h,
    the +C handled via host-side bias folds (r/z matmuls), an extra t2
    matmul pass (h), and fused scalar_tensor_tensor ops in the combine.
  - Software-pipelined emission: stage S of tile i is emitted alongside
    other stages of tiles i-1/i+1/i+2 so every engine queue always has
    ready work; the PE never idles and stays at full 2.4GHz p-state.
  - Depth-first per-chunk processing (leaf->L5 per 16-tree chunk, then a
    global L4->L0 tail) keeps all embeddings resident in SBUF.
"""

import sys

for _p in ("/opt/trn_rl_repo", "/root/.axon_site/_ro/trn_rl_repo"):
    if _p not in sys.path:
        sys.path.insert(0, _p)

import numpy as np

B = 1024
L = 10
H = 128
FEAT = 7
NCORES = 8
TPC = B // NCORES          # trees per core = 128
TCH = 16                   # trees per chunk
NCHUNK = TPC // TCH        # 8 chunks
NPC = TPC * (2 ** L - 1)   # nodes per core = 130944
LEVEL_SIZES = [B * 2 ** j for j in range(L)]
OFF = np.concatenate([[0], np.cumsum(LEVEL_SIZES)]).astype(int)
INNER = LEVEL_SIZES[:-1]
COFF = np.concatenate([[0], np.cumsum(INNER)]).astype(int)

MMT = 512  # tile size (nodes per tile)

_CACHE = {}


def _children_canonical(children):
    for j in range(L - 1):
        n = INNER[j]
        blk = children[COFF[j]:COFF[j + 1]]
        base = 2 * np.arange(n, dtype=np.int64)
        if not (np.array_equal(blk[:, 0], base) and np.array_equal(blk[:, 1], base + 1)):
            return False
    return True


def _numpy_fallback(contents, children, W_u, b_u, W_h, b_h, W_z, b_z, W_r, b_r,
                    conv_w, conv_b):
    w, b = float(conv_w[0]), float(conv_b[0])

    def conv_chain(x):
        for _ in range(3):
            x = np.maximum(w * x + b, 0.0)
        return x

    def sigmoid(x):
        return 1.0 / (1.0 + np.exp(-x))

    emb = None
    for j in reversed(range(L)):
        c = contents[OFF[j]:OFF[j + 1]]
        u = conv_chain(c @ W_u + b_u)
        if j == L - 1:
            emb = u
            continue
        ch = children[COFF[j]:COFF[j + 1]]
        h_L = emb[ch[:, 0]]
        h_R = emb[ch[:, 1]]
        hhu = np.concatenate([h_L, h_R, u], axis=1)
        r = sigmoid(hhu @ W_r + b_r)
        h_H = conv_chain((r * hhu) @ W_h + b_h)
        z = np.concatenate([h_H, hhu], axis=1) @ W_z + b_z
        zs = np.stack([z[:, :H], z[:, H:2 * H], z[:, 2 * H:3 * H], z[:, 3 * H:]], axis=-1)
        zs = zs - zs.max(axis=-1, keepdims=True)
        e = np.exp(zs)
        g = e / e.sum(axis=-1, keepdims=True)
        emb = g[..., 0] * h_H + g[..., 1] * h_L + g[..., 2] * h_R + g[..., 3] * u
    return emb.reshape(B, -1).astype(np.float32)


def make_worklist():
    """Slot list (per core). Each slot: dict(kind, level, chunk, s, n, ct_off).
    s is the chunk-local (levels>=5) or global (levels<=4) parent start.
    ct columns are laid out in exactly this slot order; leaf slots
    alternate Ev/Od halves of 1024-leaf blocks."""
    def leaf_slots(c):
        return [dict(kind="leaf", level=9, chunk=c, block=t // 2,
                     half=t % 2, n=MMT) for t in range(16)]

    def inner_slots(c, spaced=False):
        # combine2 lands 2 slots after a tile; a parent must trail its last
        # child by >= 3 slots. Zipped chunks get spacing from interleaved
        # leaves; the unzipped last chunk needs explicit bubbles.
        out = []
        for j, ntile in ((8, 8), (7, 4), (6, 2), (5, 1)):
            if spaced and j == 6:
                out.extend([dict(kind="nop"), dict(kind="nop")])
            if j == 5:
                out.append(dict(kind="nop"))
                if spaced:
                    out.extend([dict(kind="nop"), dict(kind="nop")])
            for t in range(ntile):
                out.append(dict(kind="inner", level=j, chunk=c, s=t * MMT,
                                n=MMT))
        return out

    # Phase-shifted interleave: chunk c's inner slots (DVE/Pool-heavy) zip
    # with chunk c+1's leaf slots (PE/ACT-light) — smooths per-engine load
    # and makes adjacent slots independent (doubling dependency slack,
    # incl. the tight L6 -> L5 transition).
    slots = list(leaf_slots(0))
    for c in range(NCHUNK):
        if c + 1 < NCHUNK:
            inn = inner_slots(c)
            lf = leaf_slots(c + 1)
            for i in range(16):
                if i < len(inn):
                    slots.append(inn[i])
                slots.append(lf[i])
        else:
            slots.extend(inner_slots(c, spaced=True))
    for j, ntile, n in ((4, 4, MMT), (3, 2, MMT), (2, 1, MMT), (1, 1, 256),
                        (0, 1, 128)):
        # serial tail: parent r-stage (slot i-2) must follow the last
        # child's combine2 (slot j_child+2)
        if j == 3:
            slots.extend([dict(kind="nop")] * 2)
        elif j <= 2:
            slots.extend([dict(kind="nop")] * 3)
        for t in range(ntile):
            slots.append(dict(kind="root" if j == 0 else "inner", level=j,
                              chunk=None, s=t * MMT, n=n))
    cur = 0
    for s in slots:
        if s["kind"] == "nop":
            continue
        s["ct_off"] = cur
        cur += s["n"]
    assert cur == NPC, (cur, NPC)
    return slots


def _build(cw, cb):
    from contextlib import ExitStack

    from concourse import bacc, bass, mybir, tile

    f32 = mybir.dt.float32
    bf16 = mybir.dt.bfloat16
    AF = mybir.ActivationFunctionType
    OP = mybir.AluOpType

    C = cw * cb + cb
    slots = make_worklist()

    nc = bacc.Bacc()

    ct_d = nc.declare_dram_parameter("ct", [FEAT, NPC], bf16, isOutput=False)
    wu_d = nc.declare_dram_parameter("wu", [FEAT, H], bf16, isOutput=False)
    wr_d = nc.declare_dram_parameter("wr", [H, 3, 3, H], bf16, isOutput=False)
    wh_d = nc.declare_dram_parameter("wh", [H, 4, H], bf16, isOutput=False)
    wz_d = nc.declare_dram_parameter("wz", [H, 4, 3, H], bf16, isOutput=False)
    bv_d = nc.declare_dram_parameter("bvec", [H, 9], f32, isOutput=False)
    id_d = nc.declare_dram_parameter("ident", [H, H], f32, isOutput=False)
    out_d = nc.declare_dram_parameter("out", [TPC, H], f32, isOutput=True)

    with ExitStack() as ctx:
        tc = ctx.enter_context(tile.TileContext(nc))
        wpool = ctx.enter_context(tc.tile_pool(name="wts", bufs=1))
        epool = ctx.enter_context(tc.tile_pool(name="emb", bufs=1))
        ctpool = ctx.enter_context(tc.tile_pool(name="ct", bufs=4))
        spool = ctx.enter_context(tc.tile_pool(name="scr", bufs=4))
        pp = ctx.enter_context(tc.tile_pool(name="ps", bufs=1, space="PSUM"))

        wu = wpool.tile([FEAT, H], bf16, tag="wu")
        wr = wpool.tile([H, 3, 3, H], bf16, tag="wr")
        wh = wpool.tile([H, 4, H], bf16, tag="wh")
        wz = wpool.tile([H, 4, 3, H], bf16, tag="wz")
        bv = wpool.tile([H, 9], f32, tag="bv")
        idt = wpool.tile([H, H], f32, tag="idt")
        nc.sync.dma_start(wu[:], wu_d[:])
        nc.sync.dma_start(wr[:], wr_d[:])
        nc.sync.dma_start(wh[:], wh_d[:])
        nc.sync.dma_start(wz[:], wz_d[:])
        nc.sync.dma_start(bv[:], bv_d[:])
        nc.sync.dma_start(idt[:], id_d[:])

        # ---------------- embedding buffers ----------------
        # per-chunk rotating levels 9..6; globals 5..0 (Ev | Od halves)
        CHSZ = {j: TCH * 2 ** j for j in range(5, 10)}   # per-chunk level size
        GSZ = {j: TPC * 2 ** j for j in range(0, 6)}     # global level size
        e9blk = {}    # (chunk, block) -> AP [H, 1024]
        ebuf_c = {}   # (chunk, level 8..6) -> AP
        ebuf_g = {}   # level 5..1 -> AP
        for j in range(1, 6):
            ebuf_g[j] = epool.tile([H, GSZ[j]], bf16, name=f"eg{j}", tag=f"eg{j}")
        e0 = epool.tile([H, TPC], f32, name="e0", tag="e0")

        def child_aps(slot):
            """(hL, hR) contiguous APs for an inner/root slot."""
            j, n = slot["level"], slot["n"]
            if j == 8:
                blk = e9blk[(slot["chunk"], slot["s"] // MMT)]
                return blk[:, 0:MMT], blk[:, MMT:2 * MMT]
            s = slot["s"]
            if j in (7, 6, 5):
                cbuf = ebuf_c[(slot["chunk"], j + 1)]
                half = CHSZ[j + 1] // 2
                return cbuf[:, s:s + n], cbuf[:, half + s:half + s + n]
            cbuf = ebuf_g[j + 1]
            half = GSZ[j + 1] // 2
            return cbuf[:, s:s + n], cbuf[:, half + s:half + s + n]

        def out_aps(slot):
            """(dstEv, dstOd) for the split combine write."""
            j, n = slot["level"], slot["n"]
            h = n // 2
            if j in (8, 7, 6):
                buf = ebuf_c[(slot["chunk"], j)]
                half = CHSZ[j] // 2
                s2 = slot["s"] // 2
                return buf[:, s2:s2 + h], buf[:, half + s2:half + s2 + h]
            if j == 5:
                buf = ebuf_g[5]
                half = GSZ[5] // 2
                s2 = slot["chunk"] * (CHSZ[5] // 2) + slot["s"] // 2
                return buf[:, s2:s2 + h], buf[:, half + s2:half + s2 + h]
            buf = ebuf_g[j]
            half = GSZ[j] // 2
            s2 = slot["s"] // 2
            return buf[:, s2:s2 + h], buf[:, half + s2:half + s2 + h]

        # ---------------- pipeline stages ----------------
        def st_dma(slot):
            ctt = ctpool.tile([FEAT, MMT], bf16, name="ctt", tag="ct")
            n = slot["n"]
            nc.sync.dma_start(ctt[:, :n], ct_d[:, slot["ct_off"]:slot["ct_off"] + n])
            slot["ct"] = ctt

        def st_u(slot):
            n = slot["n"]
            pu = pp.tile([H, MMT], f32, name="pu", tag="pu")
            nc.tensor.matmul(pu[:, :n], wu[:], slot["ct"][:, :n],
                             start=True, stop=True)
            if slot["kind"] == "leaf":
                c, blkidx, half = slot["chunk"], slot["block"], slot["half"]
                if half == 0:
                    # with the phase-shifted zip, up to 3 chunks' blocks can
                    # be live (written / being read / prefetched) -> 24 bufs
                    blk = spool.tile([H, 2 * MMT], bf16, name="e9b", tag="e9b",
                                     bufs=24)
                    e9blk[(c, blkidx)] = blk
                dst = e9blk[(c, blkidx)][:, half * MMT:(half + 1) * MMT]
                # emb_leaf = relu(pu + bias_u) + C  (ACT + fast 4x TS)
                nc.scalar.activation(dst, pu[:, :n], AF.Relu, bias=bv[:, 0:1])
                nc.vector.tensor_scalar(dst, dst, C, None, OP.add)
            else:
                u = spool.tile([H, MMT], bf16, name="u", tag="u", bufs=6)
                nc.scalar.activation(u[:, :n], pu[:, :n], AF.Relu,
                                     bias=bv[:, 0:1])
                slot["u"] = u

        def st_r(slot):
            n = slot["n"]
            hL, hR = child_aps(slot)
            slot["hLR"] = (hL, hR)
            u = slot["u"]
            rhs = (hL, hR, u[:, :n])
            ts = []
            for m in range(3):
                pr = pp.tile([H, MMT], f32, name="pr", tag="pr", bufs=3)
                for k in range(3):
                    nc.tensor.matmul(pr[:, :n], wr[:, k, m, :], rhs[k],
                                     start=(k == 0), stop=(k == 2))
                t = spool.tile([H, MMT], bf16, name=f"t{m}", tag=f"t{m}")
                nc.scalar.activation(t[:, :n], pr[:, :n], AF.Tanh,
                                     bias=bv[:, 1 + m:2 + m])
                ts.append(t)
            # rh_k = (t_k + 1) * x_k as fused STT on DVE (Pool has no STT
            # opcode); fresh destinations — aliased STTs measured slower.
            # t2 stays raw for the h-stage C-pass (bias colsum fold).
            rhs_x = (hL, hR, u[:, :n])
            rh = []
            for k in range(3):
                r_k = spool.tile([H, MMT], bf16, name=f"rh{k}", tag=f"rh{k}")
                nc.vector.scalar_tensor_tensor(r_k[:, :n], ts[k][:, :n], 1.0,
                                               rhs_x[k], OP.add, OP.mult)
                rh.append(r_k)
            slot["rh"] = rh
            slot["t"] = ts

        def st_h(slot):
            n = slot["n"]
            rh0, rh1, rh2 = slot["rh"]
            t2 = slot["t"][2]
            rhs = (rh0[:, :n], rh1[:, :n], rh2[:, :n], t2[:, :n])
            ph = pp.tile([H, MMT], f32, name="ph", tag="ph", bufs=2)
            for k in range(4):
                nc.tensor.matmul(ph[:, :n], wh[:, k, :], rhs[k],
                                 start=(k == 0), stop=(k == 3))
            hh = spool.tile([H, MMT], bf16, name="hh", tag="hh")
            nc.scalar.activation(hh[:, :n], ph[:, :n], AF.Relu,
                                 bias=bv[:, 4:5])
            slot["hh"] = hh

        def st_z(slot):
            n = slot["n"]
            hL, hR = slot["hLR"]
            rhs = (hL, hR, slot["hh"][:, :n], slot["u"][:, :n])
            es = []
            for m in range(3):
                pz = pp.tile([H, MMT], f32, name="pz", tag="pz", bufs=2)
                for k in range(4):
                    nc.tensor.matmul(pz[:, :n], wz[:, k, m, :], rhs[k],
                                     start=(k == 0), stop=(k == 3))
                e = spool.tile([H, MMT], bf16, name=f"e{m}", tag=f"e{m}")
                nc.scalar.activation(e[:, :n], pz[:, :n], AF.Exp,
                                     bias=bv[:, 5 + m:6 + m])
                es.append(e)
            slot["e"] = es

        def st_combine1(slot):
            n = slot["n"]
            hL, hR = slot["hLR"]
            e0t, e1t, e2t = [e[:, :n] for e in slot["e"]]
            hh = slot["hh"][:, :n]
            # S' = 1 + e0 + e1 + e2 (f32 for reciprocal; DVE has no divide)
            a = spool.tile([H, MMT], bf16, name="a", tag="a")
            nc.vector.tensor_tensor(a[:, :n], e0t, e1t, OP.add)
            Sp = spool.tile([H, MMT], f32, name="Sp", tag="Sp")
            nc.vector.scalar_tensor_tensor(Sp[:, :n], a[:, :n], 1.0, e2t,
                                           OP.add, OP.add)
            rcp = spool.tile([H, MMT], f32, name="rcp", tag="rcp")
            nc.vector.reciprocal_approx_fast(rcp[:, :n], Sp[:, :n])
            # products: p0 = e0*(hh+C) (fused STT on DVE), p1/p2 on Pool
            p0 = spool.tile([H, MMT], bf16, name="p0", tag="p0")
            nc.vector.scalar_tensor_tensor(p0[:, :n], hh, C, e0t,
                                           OP.add, OP.mult)
            nc.gpsimd.tensor_tensor(e1t, e1t, hL, OP.mult)
            nc.gpsimd.tensor_tensor(e2t, e2t, hR, OP.mult)
            slot["p0"] = p0
            slot["rcp"] = rcp

        def st_combine2(slot):
            n = slot["n"]
            e1t, e2t = slot["e"][1][:, :n], slot["e"][2][:, :n]
            u = slot["u"][:, :n]
            p0 = slot["p0"][:, :n]
            rcp = slot["rcp"]
            nc.gpsimd.tensor_tensor(e1t, p0, e1t, OP.add)
            q1 = spool.tile([H, MMT], bf16, name="q1", tag="q1")
            nc.vector.scalar_tensor_tensor(q1[:, :n], u, C, e2t,
                                           OP.add, OP.add)
            nc.gpsimd.tensor_tensor(e1t, e1t, q1[:, :n], OP.add)
            if slot["kind"] == "root":
                nc.vector.tensor_tensor(e0[:, :n], e1t, rcp[:, :n], OP.mult)
                return
            dstEv, dstOd = out_aps(slot)
            q2v = e1t.rearrange("p (n two) -> p n two", two=2)
            rcv = rcp[:, :n].rearrange("p (n two) -> p n two", two=2)
            nc.vector.tensor_tensor(dstEv, q2v[:, :, 0], rcv[:, :, 0], OP.mult)
            nc.vector.tensor_tensor(dstOd, q2v[:, :, 1], rcv[:, :, 1], OP.mult)

        # ---------------- allocate per-chunk emb tiles lazily ----------------
        def ensure_chunk_bufs(slot):
            j, c = slot["level"], slot["chunk"]
            if slot["kind"] != "leaf" and j in (8, 7, 6) and (c, j) not in ebuf_c:
                ebuf_c[(c, j)] = spool.tile([H, CHSZ[j]], bf16,
                                            name=f"ec{j}", tag=f"ec{j}", bufs=2)

        # ---------------- emission loop ----------------
        NS = len(slots)

        def get(i):
            return slots[i] if 0 <= i < NS else None

        for i in range(-4, NS + 3):
            s_dma = get(i + 4)
            s_u = get(i + 3)
            s_r = get(i + 2)
            s_h = get(i)
            s_z = get(i - 1)
            s_c2 = get(i - 2)
            if s_dma is not None and s_dma["kind"] != "nop":
                st_dma(s_dma)
            if s_z is not None and s_z["kind"] in ("inner", "root"):
                st_z(s_z)
                st_combine1(s_z)
            if s_c2 is not None and s_c2["kind"] in ("inner", "root"):
                st_combine2(s_c2)
            if s_h is not None and s_h["kind"] in ("inner", "root"):
                st_h(s_h)
            if s_u is not None and s_u["kind"] != "nop":
                ensure_chunk_bufs(s_u)
                st_u(s_u)
            if s_r is not None and s_r["kind"] in ("inner", "root"):
                st_r(s_r)

        # ---------------- output transpose + store ----------------
        pt = pp.tile([H, H], f32, name="pt", tag="pz", bufs=2)
        nc.tensor.matmul(pt[:], e0[:, :TPC], idt[:], is_transpose=True,
                         start=True, stop=True)
        osb = spool.tile([H, H], f32, name="osb", tag="osb", bufs=1)
        nc.vector.tensor_copy(osb[:], pt[:])
        nc.sync.dma_start(out_d[:], osb[:])

    nc.compile()
    if not nc.is_finalized():
        nc.finalize()
    return nc


def _prepare(inputs):
    contents = np.ascontiguousarray(np.asarray(inputs["contents"], np.float32))
    W_u = np.asarray(inputs["W_u"], np.float32)
    b_u = np.asarray(inputs["b_u"], np.float32)
    W_h = np.asarray(inputs["W_h"], np.float32)
    b_h = np.asarray(inputs["b_h"], np.float32)
    W_z = np.asarray(inputs["W_z"], np.float32)
    b_z = np.asarray(inputs["b_z"], np.float32)
    W_r = np.asarray(inputs["W_r"], np.float32)
    b_r = np.asarray(inputs["b_r"], np.float32)
    cw = float(np.asarray(inputs["conv_w"]).reshape(-1)[0])
    cb = float(np.asarray(inputs["conv_b"]).reshape(-1)[0])
    w3 = cw ** 3
    C = cw * cb + cb

    # ---- per-core, feature-major contents in worklist column order ----
    # level j rows per core, feature-major: [NCORES, FEAT, TPC*2^j]
    lev = {}
    for j in range(L):
        n = TPC * 2 ** j
        blk = contents[OFF[j]:OFF[j + 1]].reshape(NCORES, n, FEAT)
        lev[j] = blk.transpose(0, 2, 1)

    slots = make_worklist()
    cts = np.empty((NCORES, FEAT, NPC), np.float32)
    for s in slots:
        if s["kind"] == "nop":
            continue
        j, n, off = s["level"], s["n"], s["ct_off"]
        if s["kind"] == "leaf":
            c, blkidx, half = s["chunk"], s["block"], s["half"]
            base = c * TCH * 512 + blkidx * 1024
            block = lev[9][:, :, base:base + 1024]
            cts[:, :, off:off + n] = block[:, :, half::2]
        else:
            if s["chunk"] is not None:
                base = s["chunk"] * TCH * 2 ** j + s["s"]
            else:
                base = s["s"]
            cts[:, :, off:off + n] = lev[j][:, :, base:base + n]

    # ---- weights with all folds ----
    Wu_f = w3 * W_u                                   # [7, 128]
    bias_u = w3 * b_u + cw * cw * cb                  # relu_u bias
    bias_leaf = bias_u + C                            # leaf clamp bias

    # r: t_m = tanh(0.5*(hhu@W_r + b_r')) ; hhu = [hL, hR, u_r] + C on u rows
    bias_r = b_r + C * W_r[2 * H:3 * H].sum(axis=0)   # [3H]
    Wr_f = 0.5 * W_r                                  # fold tanh's 1/2
    wr_np = np.ascontiguousarray(
        Wr_f.reshape(3, H, 3, H).transpose(1, 0, 2, 3))   # [H, k, m, H]
    bias_t = 0.5 * bias_r.reshape(3, H)               # per-m tanh bias

    # h: rhs k-tiles (rh0, rh1, rh2', t2) with rh_k=(t_k+1)*x_k
    Whb = W_h.reshape(3, H, H)                        # blocks by row group
    wh_blocks = np.stack([
        0.5 * w3 * Whb[0],
        0.5 * w3 * Whb[1],
        0.5 * w3 * Whb[2],
        0.5 * w3 * C * Whb[2],
    ], axis=0)                                        # [4, H, H]
    wh_np = np.ascontiguousarray(wh_blocks.transpose(1, 0, 2))  # [H, 4, H]
    # k3 rhs is raw tanh; the C*(+1) part folds into the bias colsum
    bias_h = w3 * b_h + cw * cw * cb + 0.5 * w3 * C * Whb[2].sum(axis=0)

    # z (z3-normalized): rhs k-tiles (hL, hR, hh_r, u_r)
    Wzn = W_z[:, :3 * H].reshape(4 * H, 3, H) - W_z[:, 3 * H:].reshape(4 * H, 1, H)
    bzn = b_z[:3 * H].reshape(3, H) - b_z[3 * H:].reshape(1, H)
    # reference z rows: [0:H]=hH, [H:2H]=hL, [2H:3H]=hR, [3H:4H]=u
    k_order = [slice(H, 2 * H), slice(2 * H, 3 * H), slice(0, H),
               slice(3 * H, 4 * H)]
    wz_np = np.ascontiguousarray(
        np.stack([Wzn[ks] for ks in k_order], axis=0)      # [4, H, 3, H]
        .transpose(1, 0, 2, 3))                            # [H, 4, 3, H]
    bias_z = (bzn + C * Wzn[0:H].sum(axis=0)
              + C * Wzn[3 * H:4 * H].sum(axis=0))          # [3, H]

    bvec = np.zeros((H, 9), np.float32)
    bvec[:, 0] = bias_u
    bvec[:, 1:4] = bias_t.T
    bvec[:, 4] = bias_h
    bvec[:, 5:8] = bias_z.T
    bvec[:, 8] = bias_leaf

    import ml_dtypes

    bf = ml_dtypes.bfloat16
    common = {
        "wu": np.ascontiguousarray(Wu_f).astype(bf),
        "wr": wr_np.astype(bf), "wh": wh_np.astype(bf), "wz": wz_np.astype(bf),
        "bvec": bvec,
        "ident": np.eye(H, dtype=np.float32),
    }
    in_maps = [dict(common, ct=np.ascontiguousarray(cts[c]).astype(bf))
               for c in range(NCORES)]
    return in_maps, cw, cb


def kernel(**inputs):
    children = np.asarray(inputs["children"])
    cw = float(np.asarray(inputs["conv_w"]).reshape(-1)[0])
    cb = float(np.asarray(inputs["conv_b"]).reshape(-1)[0])
    collapsible = (cw > 0.0) and (cb >= 0.0)
    if not (collapsible and _children_canonical(children)):
        args = {k: np.asarray(v) for k, v in inputs.items()}
        return _numpy_fallback(**args)

    from concourse.bass_utils import run_bass_kernel_spmd

    key = (cw, cb)
    if key not in _CACHE:
        _CACHE[key] = _build(cw, cb)
    nc = _CACHE[key]

    in_maps, _, _ = _prepare(inputs)
    res = run_bass_kernel_spmd(nc, in_maps, list(range(NCORES)))
    outs = [res.results[c]["out"] for c in range(NCORES)]
    return np.ascontiguousarray(np.concatenate(outs, axis=0).astype(np.float32))


if __name__ == "__main__":
    print("kernel module loaded")
